# revision 29
# baseline (speedup 1.0000x reference)
"""Multi-head attention (B=16, N=1024, E=768, H=8) on 8 Trainium2 NeuronCores.

Sharding: data-parallel over batch (2 batches per core, no collectives).
Per core, one fused Tile kernel:
  - host pre-transposes x -> x^T and pre-permutes the interleaved qkv weights
    (including packing head-pair Q|K features into full 128-row chunks so the
    QK projection runs at 100% PE utilization; pieces are unscrambled into
    per-head Q^T/K^T tiles via staged adds + partition-shift SBUF DMAs)
  - V is produced per batch for all heads, 97 cols per head: a leading ones
    column (so the softmax denominator falls out of the O matmul as row 0)
    plus the 96 V columns
  - S^T = (K^T)^T @ Q^T -> PSUM, Exp on the scalar engine -> SBUF
  - O = V''^T @ exp(S^T) accumulated over key chunks, software-pipelined
    with the S matmuls (each weight load shared by both q-halves)
  - normalize with reciprocal_approx_fast (bf16) + gpsimd partition_broadcast
    + one fused DVE multiply folding the post-softmax 1/sqrt(E) scale
  - normalized O^T is repacked (partition-shift DMAs) into six full 128-row
    feature chunks so the output projection contracts over K=128 at 100% PE
    utilization; proj runs as 16 F=384 sub-tiles rotating over 6 PSUM slots
    so the tail repack latency is hidden; bias added on DVE, DMA out
Cross-batch software pipeline: batch 1's QK-gen + V-gen are emitted before
batch 0's projection so the PE never idles at phase boundaries. Dummy K=1
matmuls at t=0 warm the HAM clock gate. HBM loads are spread across the
sync/scalar/tensor/vector DMA queues; dependent SBUF-to-SBUF piece DMAs ride
the gpsimd queue and repack DMAs the vector queue (no head-of-line blocks).
"""
import sys
import os

for _p in ("/opt/trn_rl_repo", "/root/.axon_site", "/root/.axon_site/_ro/trn_rl_repo"):
    if os.path.isdir(_p) and _p not in sys.path:
        sys.path.append(_p)

import numpy as np

B, N, E, H = 16, 1024, 768, 8
D = E // H            # 96
NCORES = 8
BPC = B // NCORES     # batches per core = 2
EC = E // 128         # 6 E-chunks
TC = N // 128         # 8 token chunks
VW = D + 1            # per-head width in the V layout (ones col + 96 V cols)
SCALE = float(1.0 / np.sqrt(np.float32(E)))

PRECISION = "fast"

_NC_CACHE = {}


def _build_nc():
    import concourse.bacc as bacc
    import concourse.mybir as mybir
    import concourse.tile as tile

    FP32 = mybir.dt.float32
    FP32R = mybir.dt.float32r
    BF16 = mybir.dt.bfloat16
    DTF = BF16 if PRECISION == "fast" else FP32R
    AF = mybir.ActivationFunctionType
    OP = mybir.AluOpType

    nc = bacc.Bacc("TRN2", target_bir_lowering=False, debug=False, num_devices=NCORES)

    xt = nc.dram_tensor("xt", [BPC, 128, EC * N], DTF, kind="ExternalInput")
    wqk = nc.dram_tensor("wqk", [H // 2, 128, EC * 4 * D], DTF, kind="ExternalInput")
    wv = nc.dram_tensor("wv", [128, EC * H * VW], DTF, kind="ExternalInput")
    vb = nc.dram_tensor("vb", [1, H * VW], FP32, kind="ExternalInput")
    bqk = nc.dram_tensor("bqk", [128, (H // 2) * 3], FP32, kind="ExternalInput")
    pw = nc.dram_tensor("pw", [128, EC * E], BF16, kind="ExternalInput")
    pb = nc.dram_tensor("pb", [1, E], FP32, kind="ExternalInput")
    out = nc.dram_tensor("out", [BPC, N, E], FP32, kind="ExternalOutput")

    from contextlib import ExitStack

    with tile.TileContext(nc) as tc:
        with ExitStack() as ctx:
            const = ctx.enter_context(tc.tile_pool(name="const", bufs=1))
            xtp = ctx.enter_context(tc.tile_pool(name="xtp", bufs=2))
            vp = ctx.enter_context(tc.tile_pool(name="vp", bufs=2))
            oallp = ctx.enter_context(tc.tile_pool(name="oallp", bufs=1))
            wqp = ctx.enter_context(tc.tile_pool(name="wqp", bufs=4))
            stgp = ctx.enter_context(tc.tile_pool(name="stgp", bufs=6))
            qtp = ctx.enter_context(tc.tile_pool(name="qtp", bufs=4))
            ktp = ctx.enter_context(tc.tile_pool(name="ktp", bufs=4))
            estp = ctx.enter_context(tc.tile_pool(name="estp", bufs=4))
            rp = ctx.enter_context(tc.tile_pool(name="rp", bufs=4))
            rbcp = ctx.enter_context(tc.tile_pool(name="rbcp", bufs=2))
            rstgp = ctx.enter_context(tc.tile_pool(name="rstgp", bufs=3))
            obp = ctx.enter_context(tc.tile_pool(name="obp", bufs=3))
            qkps = ctx.enter_context(tc.tile_pool(name="qkps", bufs=2, space="PSUM"))
            stps = ctx.enter_context(tc.tile_pool(name="stps", bufs=2, space="PSUM"))
            ops = ctx.enter_context(tc.tile_pool(name="ops", bufs=2, space="PSUM"))

            # ---- resident constants ----
            bqk_sb = const.tile([128, (H // 2) * 3], FP32)
            vb_row = const.tile([1, H * VW], FP32)
            vb_sb = const.tile([128, H * VW], FP32)
            wv_sb = const.tile([128, EC * H * VW], DTF)
            pw_sb = const.tile([128, EC * E], BF16)
            pb_row = const.tile([1, E], FP32)
            pb_sb = const.tile([128, E], FP32)
            warm = const.tile([128, 512], DTF)

            # ---- prologue: loads spread across queues + HAM warmup ----
            xt_tiles = {}

            def load_xt(b):
                t = xtp.tile([128, EC * N], DTF, tag="xt", name=f"xt_{b}")
                xt_tiles[b] = t
                return t

            xt0 = load_xt(0)
            # gpsimd queue: warm-tile memset, small consts, bias broadcast
            nc.gpsimd.memset(warm[:], 0.0)
            nc.gpsimd.dma_start(bqk_sb[:], bqk.ap())
            nc.gpsimd.dma_start(vb_row[:], vb.ap())
            nc.gpsimd.partition_broadcast(vb_sb[:], vb_row[:])
            # sync queue: first QK pair weights, first x chunks, V weights
            wp_tiles = {0: wqp.tile([128, EC * 4 * D], DTF, tag="wq", name="wp_0")}
            nc.sync.dma_start(wp_tiles[0][:], wqk.ap()[0])
            # per-chunk x transfers: the first QK group starts as soon as
            # chunk 0 lands and stays fed (each chunk feeds 12 matmuls)
            for c in range(4):
                nc.sync.dma_start(xt0[:, c * N:(c + 1) * N],
                                  xt.ap()[0, :, c * N:(c + 1) * N])
            # scalar queue: last x chunks (runs concurrently with sync)
            nc.scalar.dma_start(xt0[:, 4 * N:5 * N], xt.ap()[0, :, 4 * N:5 * N])
            nc.scalar.dma_start(xt0[:, 5 * N:6 * N], xt.ap()[0, :, 5 * N:6 * N])
            nc.sync.dma_start(wv_sb[:], wv.ap())

            wmps = qkps.tile([128, 512], FP32, tag="qk", name="warm_ps")
            NWARM = 10
            for i in range(NWARM):
                # full-K dummies: the HAM activity monitor needs the whole
                # array lit to register activity and lift the clock gate
                nc.tensor.matmul(
                    wmps[0:64, :],
                    warm[:, 0:64],
                    warm[:, :],
                    start=(i == 0),
                    stop=(i == NWARM - 1),
                )

            v_tiles = {}
            oall_tiles = {}
            qt_by_head = [{} for _ in range(BPC)]
            kt_by_head = [{} for _ in range(BPC)]

            # piece table: (src_r0, src_r1, which, sub, dst_r0) per chunk
            PIECES = (
                ((0, 96, "q", 0, 0), (96, 128, "k", 0, 0)),
                ((0, 64, "k", 0, 32), (64, 128, "q", 1, 0)),
                ((0, 32, "q", 1, 64), (32, 128, "k", 1, 0)),
            )

            def emit_pair_gen(b, p):
                if p in wp_tiles:
                    wp = wp_tiles[p]
                else:
                    wp = wqp.tile([128, EC * 4 * D], DTF, tag="wq", name=f"wp_{p}")
                    wp_tiles[p] = wp
                    nc.sync.dma_start(wp[:], wqk.ap()[p])
                if p == 1 and b == 0:
                    # proj weights/bias: needed only at the projection
                    nc.sync.dma_start(pw_sb[:], pw.ap())
                    nc.sync.dma_start(pb_row[:], pb.ap())
                    nc.gpsimd.partition_broadcast(pb_sb[:], pb_row[:])
                xt_sb = xt_tiles[b]
                pq = [
                    qtp.tile([D, N], DTF, tag="qt", name=f"qt_{b}_{2 * p + i}")
                    for i in range(2)
                ]
                pk = [
                    ktp.tile([D, N], DTF, tag="kt", name=f"kt_{b}_{2 * p + i}")
                    for i in range(2)
                ]
                qt_by_head[b][2 * p] = pq[0]
                qt_by_head[b][2 * p + 1] = pq[1]
                kt_by_head[b][2 * p] = pk[0]
                kt_by_head[b][2 * p + 1] = pk[1]
                for m in range(3):
                    stg = stgp.tile([128, 1024], DTF, tag="stg",
                                    name=f"stg_{b}_{p}_{m}")
                    for qh in range(2):
                        g_ps = qkps.tile([128, 512], FP32, tag="qk",
                                         name=f"g_{b}_{p}_{m}_{qh}")
                        for c in range(EC):
                            nc.tensor.matmul(
                                g_ps[:],
                                wp[:, c * 4 * D + m * 128: c * 4 * D + (m + 1) * 128],
                                xt_sb[:, c * N + qh * 512: c * N + (qh + 1) * 512],
                                start=(c == 0),
                                stop=(c == EC - 1),
                            )
                        # on the scalar engine: it is idle during the QK
                        # window, so the g_ps PSUM slots recycle promptly
                        # (on DVE these adds queue behind the previous
                        # head's normalize chain and stall the QK matmuls)
                        nc.scalar.add(
                            stg[:, qh * 512:(qh + 1) * 512], g_ps[:],
                            bqk_sb[:, p * 3 + m: p * 3 + m + 1],
                        )
                    for r0, r1, which, psub, d0 in PIECES[m]:
                        dstt = (pq if which == "q" else pk)[psub]
                        nc.sync.dma_start(
                            dstt[d0:d0 + (r1 - r0), :],
                            stg[r0:r1, :],
                        )

            def emit_vgen(b):
                xt_sb = xt_tiles[b]
                v_sb = vp.tile([128, TC * H * VW], DTF, tag="v", name=f"v_{b}")
                v_tiles[b] = v_sb
                for t in range(TC):
                    vg = stps.tile([128, 1024], FP32, tag="st",
                                   name=f"vg_{b}_{t}")
                    for lo, hi in ((0, 512), (512, H * VW)):
                        for c in range(EC):
                            nc.tensor.matmul(
                                vg[:, lo:hi],
                                xt_sb[:, c * N + t * 128: c * N + (t + 1) * 128],
                                wv_sb[:, c * H * VW + lo: c * H * VW + hi],
                                start=(c == 0),
                                stop=(c == EC - 1),
                            )
                    nc.vector.tensor_tensor(
                        v_sb[:, t * H * VW:(t + 1) * H * VW],
                        vg[:, 0:H * VW], vb_sb[:], op=OP.add,
                    )

            def emit_head(b, h):
                qt = qt_by_head[b][h]
                kt = kt_by_head[b][h]
                v_sb = v_tiles[b]
                if b not in oall_tiles:
                    # allocated lazily: with bufs=1 the WAR hand-off against
                    # the previous batch's projection reads requires those
                    # reads to be emitted before this alloc
                    oall_tiles[b] = oallp.tile([128, EC * N], BF16, tag="oall6",
                                               name=f"oall6_{b}")
                o_all6 = oall_tiles[b]

                o_ps = [ops.tile([128, 512], FP32, tag="o",
                                 name=f"o_{b}_{h}_{i}") for i in range(2)]
                ests = [None] * TC

                def s_step(t):
                    st = stps.tile([128, 1024], FP32, tag="st")
                    for qh in range(2):
                        nc.tensor.matmul(
                            st[:, qh * 512:(qh + 1) * 512],
                            kt[:, t * 128:(t + 1) * 128],
                            qt[:, qh * 512:(qh + 1) * 512],
                            start=True,
                            stop=True,
                        )
                    est = estp.tile([128, 1024], DTF, tag="est")
                    nc.scalar.activation(est[:], st[:], AF.Exp)
                    ests[t] = est

                def o_step(t):
                    for qh in range(2):
                        nc.tensor.matmul(
                            o_ps[qh][0:VW, :],
                            v_sb[:, t * H * VW + h * VW: t * H * VW + (h + 1) * VW],
                            ests[t][:, qh * 512:(qh + 1) * 512],
                            start=(t == 0),
                            stop=(t == TC - 1),
                        )

                LAT = 2
                for t in range(TC):
                    s_step(t)
                    if t >= LAT:
                        o_step(t - LAT)
                for t in range(TC - LAT, TC):
                    o_step(t)

                # normalize into a [97, 1024] bf16 stage, then partition-shift
                # DMA the 96 V rows into the packed 6x128 projection layout
                # (per q-half, so the last transfer starts as early as possible)
                stage = rstgp.tile([VW, 1024], BF16, tag="rstg",
                                   name=f"stage_{b}_{h}")
                g0 = h * D
                c0, p0 = divmod(g0, 128)
                n1 = min(D, 128 - p0)
                # both recips first so the two gpsimd broadcasts pipeline
                # instead of ping-ponging with the STTs
                rs, rbcs = [], []
                for qh in range(2):
                    r = rp.tile([1, 512], FP32, tag="r")
                    nc.vector.reciprocal_approx_fast(r[:], o_ps[qh][0:1, :])
                    rs.append(r)
                for qh in range(2):
                    rbc = rbcp.tile([VW, 512], FP32, tag="rbc")
                    nc.gpsimd.partition_broadcast(rbc[:], rs[qh][:])
                    rbcs.append(rbc)
                for qh in range(2):
                    nc.vector.scalar_tensor_tensor(
                        stage[:, qh * 512:(qh + 1) * 512],
                        o_ps[qh][0:VW, :],
                        SCALE,
                        rbcs[qh][:],
                        OP.mult,
                        OP.mult,
                    )
                    lo = qh * 512
                    nc.sync.dma_start(
                        o_all6[p0:p0 + n1, c0 * N + lo: c0 * N + lo + 512],
                        stage[1:1 + n1, lo:lo + 512],
                    )
                    if n1 < D:
                        nc.sync.dma_start(
                            o_all6[0:D - n1, (c0 + 1) * N + lo:(c0 + 1) * N + lo + 512],
                            stage[1 + n1:1 + D, lo:lo + 512],
                        )

            def emit_proj(b):
                # 16 sub-tiles (token-chunk x 384-col half) rotating over 6
                # one-bank PSUM slots across the three pools: the 6-deep
                # rotation keeps the PE busy on chunks 0-4 while the last
                # head's repack DMA lands
                o_all6 = oall_tiles[b]
                subs = []
                for t in range(TC):
                    for colh in range(2):
                        subs.append((t, colh))
                # qk slots first: they have been free since the last pair-gen,
                # while st/o slots are released by the last head's exps/STTs
                pools = (qkps, stps, ops)
                tags = ("qk", "st", "o")
                ob_tiles = {}
                pj_tiles = {}
                DEPTH = 5

                def open_sub(i):
                    t, colh = subs[i]
                    pool = pools[i % 3]
                    pj = pool.tile([128, 384], FP32, tag=tags[i % 3],
                                   name=f"pj_{b}_{t}_{colh}")
                    pj_tiles[i] = pj
                    e0 = colh * 384
                    for c in range(EC - 1):
                        nc.tensor.matmul(
                            pj[:],
                            o_all6[:, c * N + t * 128: c * N + (t + 1) * 128],
                            pw_sb[:, c * E + e0: c * E + e0 + 384],
                            start=(c == 0),
                            stop=False,
                        )

                def close_sub(i):
                    # the last-chunk matmul trails DEPTH sub-tiles behind so
                    # the in-order engine queue has chunk-0..4 work to run
                    # while the final head's repack DMA lands
                    t, colh = subs[i]
                    pj = pj_tiles.pop(i)
                    e0 = colh * 384
                    c = EC - 1
                    nc.tensor.matmul(
                        pj[:],
                        o_all6[:, c * N + t * 128: c * N + (t + 1) * 128],
                        pw_sb[:, c * E + e0: c * E + e0 + 384],
                        start=False,
                        stop=True,
                    )
                    if t not in ob_tiles:
                        ob_tiles[t] = obp.tile([128, E], FP32, tag="ob",
                                               name=f"ob_{b}_{t}")
                    ob = ob_tiles[t]
                    nc.vector.tensor_tensor(
                        ob[:, e0:e0 + 384], pj[:], pb_sb[:, e0:e0 + 384],
                        op=OP.add,
                    )
                    if colh == 1:
                        # one contiguous 3KB-per-token store per token chunk,
                        # issued from the otherwise-idle scalar queue
                        nc.scalar.dma_start(
                            out.ap()[b, t * 128:(t + 1) * 128, :], ob[:]
                        )

                for i in range(len(subs)):
                    open_sub(i)
                    if i >= DEPTH:
                        close_sub(i - DEPTH)
                for i in range(len(subs) - DEPTH, len(subs)):
                    close_sub(i)

            # ---- schedule: cross-batch software pipeline; each QK pair is
            # generated one head early so its unscramble DMAs land before
            # the S matmuls need them ----
            for b in range(BPC):
                for h in range(H):
                    if h == 0:
                        if 0 not in qt_by_head[b]:
                            emit_pair_gen(b, 0)
                        if b == 0:
                            emit_vgen(b)
                            # prefetch the remaining QK pair weights so they
                            # are never queued behind the big x transfers
                            for p in range(1, H // 2):
                                wp = wqp.tile([128, EC * 4 * D], DTF, tag="wq",
                                              name=f"wp_{p}")
                                wp_tiles[p] = wp
                                nc.sync.dma_start(wp[:], wqk.ap()[p])
                    if h % 2 == 1 and h < H - 1:
                        emit_pair_gen(b, (h + 1) // 2)
                    if h == 4 and b + 1 < BPC:
                        t_next = load_xt(b + 1)
                        nc.sync.dma_start(t_next[:], xt.ap()[b + 1])
                    emit_head(b, h)
                if b + 1 < BPC:
                    # front of the next batch before this batch's projection:
                    # covers the last head's repack latency and keeps the
                    # sync queue free of inverted waits
                    emit_pair_gen(b + 1, 0)
                    emit_vgen(b + 1)
                emit_proj(b)

    nc.compile()
    return nc


def get_nc():
    if "nc" not in _NC_CACHE:
        _NC_CACHE["nc"] = _build_nc()
    return _NC_CACHE["nc"]


def _prep_inputs(x, qkv_w, qkv_b, proj_w, proj_b):
    """Host-side layout prep shared by all cores + per-core x shards."""
    x = np.ascontiguousarray(x, dtype=np.float32)
    qkv_w = np.asarray(qkv_w, dtype=np.float32)
    qkv_b = np.asarray(qkv_b, dtype=np.float32)
    proj_w = np.asarray(proj_w, dtype=np.float32)
    proj_b = np.asarray(proj_b, dtype=np.float32)

    hh = np.arange(H)[:, None]
    dd = np.arange(D)[None, :]
    idx = [(hh * 3 * D + dd * 3 + c).reshape(-1) for c in range(3)]  # [768] each

    import ml_dtypes
    dtf = ml_dtypes.bfloat16 if PRECISION == "fast" else np.float32
    # packed head-pair QK weights: [H/2, 128, EC*4D]; per E-chunk the 384
    # feature cols are [Q_2p (96) | K_2p (96) | Q_2p+1 (96) | K_2p+1 (96)]
    wqT = qkv_w[idx[0], :].T.reshape(EC, 128, H, D)  # [c, p, h, d]
    wkT = qkv_w[idx[1], :].T.reshape(EC, 128, H, D)
    wqk_l = np.empty((H // 2, 128, EC, 4, D), dtype=np.float32)
    for pr in range(H // 2):
        wqk_l[pr, :, :, 0, :] = wqT[:, :, 2 * pr, :].transpose(1, 0, 2)
        wqk_l[pr, :, :, 1, :] = wkT[:, :, 2 * pr, :].transpose(1, 0, 2)
        wqk_l[pr, :, :, 2, :] = wqT[:, :, 2 * pr + 1, :].transpose(1, 0, 2)
        wqk_l[pr, :, :, 3, :] = wkT[:, :, 2 * pr + 1, :].transpose(1, 0, 2)
    wqk_l = np.ascontiguousarray(wqk_l.reshape(H // 2, 128, EC * 4 * D).astype(dtf))

    # wv: [128, EC*H*VW]; ones col at d=0 per head
    wvT = qkv_w[idx[2], :].T.reshape(EC, 128, H, D)  # [c, p, h, d]
    wv_l = np.zeros((128, EC, H, VW), dtype=np.float32)
    wv_l[:, :, :, 1:D + 1] = wvT.transpose(1, 0, 2, 3)
    wv_l = np.ascontiguousarray(wv_l.reshape(128, EC * H * VW).astype(dtf))

    # vb: [1, H*VW] v-bias row + ones column at d=0 (broadcast on device)
    vb_row = np.zeros((H, VW), dtype=np.float32)
    vb_row[:, 1:D + 1] = qkv_b[idx[2]].reshape(H, D)
    vb_row[:, 0] = 1.0
    vb_l = np.ascontiguousarray(vb_row.reshape(1, H * VW))

    # bqk: [128, 3*H/2]; col p*3+m = per-partition bias for packed chunk m
    bq = qkv_b[idx[0]].reshape(H, D)
    bk = qkv_b[idx[1]].reshape(H, D)
    bqk_l = np.zeros((128, (H // 2) * 3), dtype=np.float32)
    for pr in range(H // 2):
        bqk_l[0:96, pr * 3 + 0] = bq[2 * pr]
        bqk_l[96:128, pr * 3 + 0] = bk[2 * pr][0:32]
        bqk_l[0:64, pr * 3 + 1] = bk[2 * pr][32:96]
        bqk_l[64:128, pr * 3 + 1] = bq[2 * pr + 1][0:64]
        bqk_l[0:32, pr * 3 + 2] = bq[2 * pr + 1][64:96]
        bqk_l[32:128, pr * 3 + 2] = bk[2 * pr + 1][0:96]

    # pw: packed 6x128 layout; pw_l[p, c*E+e] = proj_w[e, c*128+p]
    pw_l = np.ascontiguousarray(
        proj_w.T.reshape(EC, 128, E).transpose(1, 0, 2).reshape(128, EC * E)
    ).astype(ml_dtypes.bfloat16)

    pb_l = np.ascontiguousarray(proj_b.reshape(1, E))

    # x^T per batch in sbuf layout: [B, 128, EC*N]; [b, p, c*N+n] = x[b, n, c*128+p]
    xt_all = np.ascontiguousarray(
        x.reshape(B, N, EC, 128).transpose(0, 3, 2, 1).reshape(B, 128, EC * N)
    ).astype(dtf)

    in_maps = []
    for core in range(NCORES):
        xt_core = np.ascontiguousarray(
            xt_all[core * BPC:(core + 1) * BPC]
        )
        in_maps.append(
            {
                "xt": xt_core,
                "wqk": wqk_l,
                "wv": wv_l,
                "vb": vb_l,
                "bqk": bqk_l,
                "pw": pw_l,
                "pb": pb_l,
            }
        )
    return in_maps


def run(inputs, trace=False):
    from concourse.bass_utils import run_bass_kernel_spmd

    nc = get_nc()
    in_maps = _prep_inputs(**inputs)
    res = run_bass_kernel_spmd(
        nc, in_maps, core_ids=list(range(NCORES)), trace=trace
    )
    out = np.concatenate([res.results[c]["out"] for c in range(NCORES)], axis=0)
    return out, res


def kernel(**inputs) -> np.ndarray:
    out, _ = run(inputs, trace=False)
    return out
